# revision 1
# baseline (speedup 1.0000x reference)
"""Differential attention kernel for Trainium2, 8-core SPMD.

Problem: B=2, S=2048, D=1024, 16 heads x 64 head-dim differential attention
(two softmaxes, combined with a scalar lambda), with input/output projections.

Sharding: data-parallel over batch (2 groups of 4 cores) x tensor-parallel
over heads (4 heads per core). Each core computes q/k/v projections for its
4 heads, both attention softmaxes, and a partial output projection
(its heads' rows of Wo). Host sums the 4 partial outputs per batch, adds bo.

v2 layout notes (vs the earlier transposed-PV version):
  - Scores are computed transposed, sT[k, q] in [128, 2*QB] PSUM tiles
    (branch 1 cols 0:QB, branch 2 cols QB:2QB); exp runs on ACT straight out
    of PSUM with the mask folded into the per-partition bias. ACT does exp
    ONLY (no DMA issue, no copies) - it is the bottleneck engine at
    ~1.04us per [128, 1024] tile.
  - PV runs "transposed": et chunks are the stationary operand
    ([128k, 128q] slices) and v_aug the moving operand ([128k, 65]), so the
    output is [128q, 65] at a cost of 65 rows instead of 512. PV cycles drop
    2x vs the [65, 512] orientation. Accumulators for 4 q-blocks live at
    128-col offsets of a [128, 512] PSUM bank tile, zeroed by DVE memset and
    accumulated with start=False (PSUM zero-regions are bank-granular, so
    start=True on a shared bank would clobber sibling slots).
  - Softmax sums arrive via the appended ones-column (col 64 of each slot);
    normalization uses native per-partition tensor_scalar ops - no DRAM
    bounce / broadcast DMAs.
  - stg ([q, hd] per head) is rebuilt to [hd, q] via PE transpose (identity
    moving operand) in [128, 128] head-pair blocks, then the out-projection
    contracts K=128 head-pairs (2 matmuls per [128d, 512q] tile).
All matmuls run in bf16 with fp32 PSUM accumulation; output partials ship as
fp16 and are reduced across cores in fp32 on the host.
"""

import sys

sys.path.insert(0, "/opt/trn_rl_repo")

from contextlib import ExitStack

import ml_dtypes
import numpy as np

import concourse.bacc as bacc
import concourse.tile as tile
from concourse import mybir
from concourse.bass_utils import run_bass_kernel_spmd

B, S, D = 2, 2048, 1024
NH, HD = 16, 64
NCORES = 8
HPC = 4              # heads per core
QB = 512             # q block (free dim of score matmuls)
NJ = S // QB         # 4
KC = 128             # k chunk (partition dim of transposed scores)
NKC = S // KC        # 16
NDI = D // 128       # 8 contraction chunks for projections
VA = HD + 1          # v columns per head incl. ones column

BF16 = mybir.dt.bfloat16
F16 = mybir.dt.float16
F32 = mybir.dt.float32
npbf16 = ml_dtypes.bfloat16

# Module-level cache: the Bass module depends only on shapes and lambda.
_BUILD_CACHE = {}
TRACE = False
LAST_RESULTS = None


def _build(lam: float, with_bias: bool = True, repeat: int = 1):
    nc = bacc.Bacc(None, target_bir_lowering=False)

    hst_d = nc.dram_tensor("hst", [D, S], BF16, kind="ExternalInput")
    wq_d = nc.dram_tensor("wq", [D, 2 * HPC * HD], BF16, kind="ExternalInput")
    wk_d = nc.dram_tensor("wk", [D, 2 * HPC * HD], BF16, kind="ExternalInput")
    wv_d = nc.dram_tensor("wv", [D, HPC * HD], BF16, kind="ExternalInput")
    wo_d = nc.dram_tensor("wo", [HPC * HD, D], BF16, kind="ExternalInput")
    bq_d = nc.dram_tensor("bq", [1, 2 * HPC * HD], BF16, kind="ExternalInput")
    bk_d = nc.dram_tensor("bk", [1, 2 * HPC * HD], BF16, kind="ExternalInput")
    bv_d = nc.dram_tensor("bv", [1, HPC * HD], BF16, kind="ExternalInput")
    mask_d = nc.dram_tensor("maskc", [KC, NKC], F32, kind="ExternalInput")
    id_d = nc.dram_tensor("ident", [128, 128], BF16, kind="ExternalInput")
    out_d = nc.dram_tensor("outT", [D, S], F16, kind="ExternalOutput")

    with tile.TileContext(nc) as tc, ExitStack() as ctx:
        const = ctx.enter_context(tc.tile_pool(name="const", bufs=1))
        wpool = ctx.enter_context(tc.tile_pool(name="wpool", bufs=1))
        hpool = ctx.enter_context(tc.tile_pool(name="hpool", bufs=1))
        qkpool = ctx.enter_context(tc.tile_pool(name="qkpool", bufs=1))
        vpool = ctx.enter_context(tc.tile_pool(name="vpool", bufs=1))
        epool = ctx.enter_context(tc.tile_pool(name="epool", bufs=28))
        rpool = ctx.enter_context(tc.tile_pool(name="rpool", bufs=3))
        tpool = ctx.enter_context(tc.tile_pool(name="tpool", bufs=4))
        slabp = ctx.enter_context(tc.tile_pool(name="slabp", bufs=1))
        sgpool = ctx.enter_context(tc.tile_pool(name="sgpool", bufs=1))
        opool = ctx.enter_context(tc.tile_pool(name="opool", bufs=4))
        ps_sc = ctx.enter_context(tc.tile_pool(name="ps_sc", bufs=2, space="PSUM"))
        ps_pv = ctx.enter_context(tc.tile_pool(name="ps_pv", bufs=2, space="PSUM"))
        ps_tr = ctx.enter_context(tc.tile_pool(name="ps_tr", bufs=2, space="PSUM"))

        # ---- constants and weights in SBUF. DMAs are consolidated (few big
        # strided transfers: HWDGE generation is ~625ns serial per DMA) and
        # ordered so the first-exp dependency chain (mask, wq/wk, hs j0)
        # lands first. Chunk c of a folded weight lives at a column offset.
        maskt = const.tile([KC, NKC], F32, tag="mask")
        nc.sync.dma_start(out=maskt[:], in_=mask_d[:])

        def load_folded(pool, dram, rows, width, tag, nsplit=2, eng=None):
            # [rows, width] DRAM -> [128, (rows/128) * width] SBUF, chunk-major
            nch = rows // 128
            t = pool.tile([128, nch * width], BF16, tag=tag)
            step = nch // nsplit
            for s in range(nsplit):
                src = dram[s * step * 128:(s + 1) * step * 128, :]
                (eng or nc.sync).dma_start(
                    out=t[:, s * step * width:(s + 1) * step * width]
                        .rearrange("p (c w) -> p c w", w=width),
                    in_=src.rearrange("(c p) w -> p c w", p=128),
                )
            return t

        hstj = [None] * NJ
        hstj[0] = load_folded(hpool, hst_d[:, 0:QB], D, QB, "hs0",
                              eng=nc.gpsimd)
        wqt = load_folded(wpool, wq_d, D, 2 * HPC * HD, "wq")
        wkt = load_folded(wpool, wk_d, D, 2 * HPC * HD, "wk", eng=nc.gpsimd)
        wvt = load_folded(wpool, wv_d, D, HPC * HD, "wv", nsplit=1)
        for j in range(1, NJ):
            hstj[j] = load_folded(hpool, hst_d[:, j * QB:(j + 1) * QB], D, QB,
                                  f"hs{j}")
        wot = load_folded(wpool, wo_d, HPC * HD, D, "wo", nsplit=1)
        ident = const.tile([128, 128], BF16, tag="ident")
        nc.gpsimd.dma_start(out=ident[:], in_=id_d[:])
        bqt = const.tile([1, 2 * HPC * HD], BF16, tag="bq")
        nc.gpsimd.dma_start(out=bqt[:], in_=bq_d[:])
        bkt = const.tile([1, 2 * HPC * HD], BF16, tag="bk")
        nc.gpsimd.dma_start(out=bkt[:], in_=bk_d[:])
        bvt = const.tile([1, HPC * HD], BF16, tag="bv")
        nc.gpsimd.dma_start(out=bvt[:], in_=bv_d[:])
        ones = const.tile([1, S], BF16, tag="ones")
        nc.gpsimd.memset(ones[:], 1.0)

        WQW = 2 * HPC * HD  # column stride per chunk in wqt/wkt
        WVW = HPC * HD

        def emit_qk_proj_j(h, j):
            # qT/kT for head h, block j: psum [128, QB] ([q1;q2] stacked on
            # partitions) accumulated over 8 di-chunks (+ K=1 bias matmul),
            # evicted (cast) to bf16.
            for wt, bt, dsts in ((wqt, bqt, qt), (wkt, bkt, kt)):
                lo = h * 128
                ps = ps_tr.tile([128, QB], F32, tag="tr")
                for c in range(NDI):
                    nc.tensor.matmul(
                        ps[:],
                        lhsT=wt[:, c * WQW + lo:c * WQW + lo + 128],
                        rhs=hstj[j][:, c * QB:(c + 1) * QB],
                        start=(c == 0),
                        stop=(not with_bias and c == NDI - 1),
                    )
                if with_bias:
                    nc.tensor.matmul(
                        ps[:],
                        lhsT=bt[0:1, lo:lo + 128],
                        rhs=ones[0:1, j * QB:(j + 1) * QB],
                        start=False,
                        stop=True,
                    )
                nc.vector.tensor_copy(dsts[h][j][:], ps[:])

        def emit_v_proj_chunk(sc):
            # v[s, 4*64] for s-chunk sc, scattered into v_aug (65-wide head
            # blocks, ones column preset by memset).
            ps = ps_tr.tile([128, HPC * HD], F32, tag="tr")
            for c in range(NDI):
                nc.tensor.matmul(
                    ps[:],
                    lhsT=hstj[sc // 4][:, c * QB + (sc % 4) * 128:
                                       c * QB + (sc % 4 + 1) * 128],
                    rhs=wvt[:, c * WVW:(c + 1) * WVW],
                    start=(c == 0),
                    stop=(not with_bias and c == NDI - 1),
                )
            if with_bias:
                nc.tensor.matmul(
                    ps[:],
                    lhsT=ones[0:1, 0:128],
                    rhs=bvt[0:1, :],
                    start=False,
                    stop=True,
                )
            src = ps[:].rearrange("p (h x) -> p h x", x=HD)
            dst = va[sc][:].rearrange("p (h y) -> p h y", y=VA)[:, :, 0:HD]
            nc.vector.tensor_copy(dst, src)

        NQB = QB // 128  # 4 q sub-blocks per j

        # Pipelined emission: deferred work (PV quads, norm, transposes,
        # out-proj groups, next-head projections) sits in a FIFO and drains
        # between score-matmul halves of the CURRENT stream. Quads are 4
        # matmuls (= PE wait-queue depth), so a quad whose exp has not
        # retired parks in the wait queue while later scores execute around
        # it; window tails drain inside the next window, so ACT never sees
        # a boundary gap. RESERVE keeps ~2.5 chunks of backlog so drained
        # quads' exps have retired.
        pending = []
        p_head = [0]
        RESERVE = 5

        def drain(nmax):
            done = 0
            while done < nmax and len(pending) - p_head[0] > RESERVE:
                pending[p_head[0]]()
                p_head[0] += 1
                done += 1

        def flush_pending():
            while p_head[0] < len(pending):
                pending[p_head[0]]()
                p_head[0] += 1

        def emit_attn(j, h, per_chunk=None):
            pvs = []
            for br in range(2):
                pv = ps_pv.tile([128, QB], F32, tag="pv", name=f"pv{j}_{h}_{br}")
                nc.vector.memset(pv[:], 0.0)
                pvs.append(pv)

            def make_quad(c, br, et):
                def quad():
                    for qb in range(NQB):
                        nc.tensor.matmul(
                            pvs[br][:, qb * 128:qb * 128 + VA],
                            lhsT=et[:, br * QB + qb * 128:br * QB + (qb + 1) * 128],
                            rhs=va[c][:, h * VA:(h + 1) * VA],
                            start=False,
                            stop=(c == NKC - 1),
                            skip_group_check=True,
                        )
                return quad

            for c in range(NKC):
                sp = ps_sc.tile([128, 2 * QB], F32, tag="sp")
                kj, kcol = divmod(c * KC, QB)
                nc.tensor.matmul(
                    sp[:, 0:QB],
                    lhsT=kt[h][kj][0:64, kcol:kcol + KC],
                    rhs=qt[h][j][0:64, :],
                    start=True,
                    stop=True,
                )
                drain(1 if len(pending) - p_head[0] < 20 else 2)
                nc.tensor.matmul(
                    sp[:, QB:2 * QB],
                    lhsT=kt[h][kj][64:128, kcol:kcol + KC],
                    rhs=qt[h][j][64:128, :],
                    start=True,
                    stop=True,
                )
                et = epool.tile([128, 2 * QB], BF16, tag="et")
                nc.scalar.activation(
                    et[:],
                    sp[:],
                    mybir.ActivationFunctionType.Exp,
                    bias=maskt[:, c:c + 1],
                    scale=float(HD) ** -0.5,
                )
                drain(1 if len(pending) - p_head[0] < 20 else 2)
                if per_chunk is not None:
                    per_chunk(c)
                pending.append(make_quad(c, 0, et))
                pending.append(make_quad(c, 1, et))
            pending.append(lambda: emit_norm(j, h, pvs))

        def emit_norm(j, h, pvs):
            pv1, pv2 = pvs
            # normalization: out = pv1/r1 - lam * pv2/r2, with sums at col
            # 64 of each 128-col slot; per-partition scalars via
            # tensor_scalar / scalar_tensor_tensor.
            rz = rpool.tile([128, 3 * NQB], F32, tag="rz")
            sums1 = pv1[:].rearrange("p (q c) -> p q c", c=128)[:, :, VA - 1:VA]
            sums2 = pv2[:].rearrange("p (q c) -> p q c", c=128)[:, :, VA - 1:VA]
            nc.vector.reciprocal(out=rz[:, 0:NQB], in_=sums1)
            nc.vector.reciprocal(out=rz[:, NQB:2 * NQB], in_=sums2)
            nc.vector.tensor_scalar_mul(
                rz[:, 2 * NQB:3 * NQB], rz[:, NQB:2 * NQB], float(-lam)
            )
            hp, hh = divmod(h, 2)
            for qb in range(NQB):
                t1 = tpool.tile([128, HD], F32, tag="t1")
                nc.vector.tensor_scalar_mul(
                    t1[:], pv1[:, qb * 128:qb * 128 + HD], rz[:, qb:qb + 1]
                )
                nc.vector.scalar_tensor_tensor(
                    out=slab[j][qb][hp][:, hh * HD:(hh + 1) * HD],
                    in0=pv2[:, qb * 128:qb * 128 + HD],
                    scalar=rz[:, 2 * NQB + qb:2 * NQB + qb + 1],
                    in1=t1[:],
                    op0=mybir.AluOpType.mult,
                    op1=mybir.AluOpType.add,
                )

        def emit_transpose(j, hp):
            # stg [q, hd-pair] -> [hd-pair, q] via PE transpose, evict into
            # the stg_pair tile consumed by the out-projection.
            for qb in range(NQB):
                tp = ps_tr.tile([128, 128], BF16, tag="tr", name=f"tp{j}_{hp}_{qb}")
                nc.tensor.matmul(
                    tp[:],
                    lhsT=slab[j][qb][hp][:],
                    rhs=ident[:],
                    is_transpose=True,
                    start=True,
                    stop=True,
                )
                nc.vector.tensor_copy(
                    stg_pair[j][hp][:, qb * 128:(qb + 1) * 128], tp[:]
                )

        def emit_outproj_d(j, d):
            # partial out-projection: outT[do, qblock] = sum_hp wo_hp.T @ stg_hp
            ps = ps_tr.tile([128, QB], F32, tag="tr", name=f"op{j}_{d}")
            for hp in range(HPC // 2):
                nc.tensor.matmul(
                    ps[:],
                    lhsT=wot[:, hp * D + d * 128:hp * D + (d + 1) * 128],
                    rhs=stg_pair[j][hp][:],
                    start=(hp == 0),
                    stop=(hp == HPC // 2 - 1),
                )
            ot = opool.tile([128, QB], F16, tag="ot")
            nc.vector.tensor_copy(ot[:], ps[:])
            nc.sync.dma_start(
                out=out_d[d * 128:(d + 1) * 128, j * QB:(j + 1) * QB],
                in_=ot[:],
            )

        # ---- emission order: heads outer so ACT streams without gaps;
        # v-proj and head h+1's projections fill PE slack during attention;
        # transposes run after each head-pair, out-projection per j as soon
        # as the last head's stage lands (only j=NJ-1 is a true tail).
        for _rep in range(repeat):
            qt = [[qkpool.tile([128, QB], BF16, tag=f"qt{h}_{j}", name=f"qt{h}_{j}")
                   for j in range(NJ)] for h in range(HPC)]
            kt = [[qkpool.tile([128, QB], BF16, tag=f"kt{h}_{j}", name=f"kt{h}_{j}")
                   for j in range(NJ)] for h in range(HPC)]
            va = [vpool.tile([128, HPC * VA], BF16, tag=f"va{c}", name=f"va{c}")
                  for c in range(NKC)]
            slab = [[[slabp.tile([128, 2 * HD], BF16, tag=f"sl{j}_{qb}_{hp}",
                                 name=f"sl{j}_{qb}_{hp}")
                      for hp in range(HPC // 2)] for qb in range(NQB)]
                    for j in range(NJ)]
            stg_pair = [[sgpool.tile([128, QB], BF16, tag=f"sg{j}_{hp}",
                                     name=f"sg{j}_{hp}")
                         for hp in range(HPC // 2)] for j in range(NJ)]
            for c in range(NKC):
                nc.gpsimd.memset(va[c][:], 1.0)

            # Only proj(0, 0) runs before the first scores; the remaining
            # head-0 projections are injected just-in-time inside window
            # (0, 0) (scores chunk c reads kt[0][c // 4], so proj(0, kj)
            # must precede chunk 4*kj), as are the v-proj chunks (pvt(c)
            # needs va[c], emitted with LAG >= 3 after chunk c). Head h+1's
            # groups then spread one-per-window across head h's row.
            emit_qk_proj_j(0, 0)

            def first_window_jit(c):
                pending.append(lambda c=c: emit_v_proj_chunk(c))
                if c in (3, 7, 11):
                    emit_qk_proj_j(0, c // 4 + 1)

            for h in range(HPC):
                for j in range(NJ):
                    emit_attn(j, h,
                              per_chunk=first_window_jit if (h, j) == (0, 0)
                              else None)
                    if h % 2 == 1:
                        pending.append(
                            lambda j=j, hp=h // 2: emit_transpose(j, hp))
                    if h == HPC - 1:
                        for d in range(NDI):
                            pending.append(lambda j=j, d=d: emit_outproj_d(j, d))
                    if h < HPC - 1:
                        pending.append(
                            lambda h=h, j=j: emit_qk_proj_j(h + 1, j))
            flush_pending()

    nc.compile()
    return nc


def _prep_inputs(hidden_states, attention_mask, Wq, bq, Wk, bk, Wv, bv, Wo):
    """Build the 8 per-core input maps (host-side shard + transpose + cast)."""
    in_maps = []
    hsT = [np.ascontiguousarray(hidden_states[b].T).astype(npbf16) for b in range(B)]
    maskc = [
        np.ascontiguousarray(
            ((1.0 - attention_mask[b]) * -10000.0).astype(np.float32).reshape(NKC, KC).T
        )
        for b in range(B)
    ]
    ident = np.eye(128, dtype=npbf16)
    for core in range(NCORES):
        b = core // (NCORES // B)
        hb = (core % (NCORES // B)) * HPC
        heads = range(hb, hb + HPC)
        qk_idx = np.concatenate(
            [np.r_[h * HD:(h + 1) * HD, D + h * HD:D + (h + 1) * HD] for h in heads]
        )
        v_idx = np.r_[hb * HD:(hb + HPC) * HD]
        in_maps.append(
            {
                "hst": hsT[b],
                "wq": np.ascontiguousarray(Wq[:, qk_idx]).astype(npbf16),
                "wk": np.ascontiguousarray(Wk[:, qk_idx]).astype(npbf16),
                "wv": np.ascontiguousarray(Wv[:, v_idx]).astype(npbf16),
                "wo": np.ascontiguousarray(Wo[v_idx, :]).astype(npbf16),
                "bq": bq[qk_idx].reshape(1, -1).astype(npbf16),
                "bk": bk[qk_idx].reshape(1, -1).astype(npbf16),
                "bv": bv[v_idx].reshape(1, -1).astype(npbf16),
                "maskc": maskc[b],
                "ident": ident,
            }
        )
    return in_maps


def kernel(
    hidden_states,
    attention_mask,
    Wq,
    bq,
    Wk,
    bk,
    Wv,
    bv,
    Wo,
    bo,
    lq1,
    lk1,
    lq2,
    lk2,
):
    global LAST_RESULTS
    args = [hidden_states, attention_mask, Wq, bq, Wk, bk, Wv, bv, Wo, bo]
    hidden_states, attention_mask, Wq, bq, Wk, bk, Wv, bv, Wo, bo = (
        np.asarray(a, dtype=np.float32) for a in args
    )
    lq1, lk1, lq2, lk2 = (np.asarray(a, dtype=np.float64) for a in (lq1, lk1, lq2, lk2))
    lam = float(np.exp(lq1 @ lk1) - np.exp(lq2 @ lk2) + 0.8)

    with_bias = not (
        np.all(bq == 0) and np.all(bk == 0) and np.all(bv == 0)
    )
    key = (round(lam, 9), with_bias)
    if key not in _BUILD_CACHE:
        _BUILD_CACHE.clear()
        _BUILD_CACHE[key] = _build(lam, with_bias)
    nc = _BUILD_CACHE[key]

    in_maps = _prep_inputs(hidden_states, attention_mask, Wq, bq, Wk, bk, Wv, bv, Wo)
    res = run_bass_kernel_spmd(nc, in_maps, core_ids=list(range(NCORES)), trace=TRACE)
    LAST_RESULTS = res

    out = np.empty((B, S, D), dtype=np.float32)
    gpb = NCORES // B
    for b in range(B):
        acc = res.results[b * gpb]["outT"].astype(np.float32)
        for g in range(1, gpb):
            acc = acc + res.results[b * gpb + g]["outT"]
        out[b] = acc.T + bo[None, :]
    return out

